# revision 19
# baseline (speedup 1.0000x reference)
"""Trainium2 Bass kernel for nn_DegreePrediction (RBC via batched Perron vectors).

Math: M[s,t] = weights_r*r_zeros + r_const is positive column-stochastic
(columns sum to 1), so its eigenvalue-1 right eigenvector is the Perron
vector, and the reference formula  rbc[n] = sum_{s,t} T[s,t]/v[s,t,s] * v[s,t,n]
is invariant to the scale of v.  rowsums(M^2) = M^2 @ 1 converges to v at
rate lambda2^2 (lambda2 <= ~0.09 here); in bf16 the end-to-end error is
~1.6e-3, well inside the 2e-2 gate, so ONE bf16 64x64x64 matmul per pair
suffices.

Sharding: the 4096 (s,t) pairs are split by s across 8 cores (512 pairs
each).  Each core computes a partial 64-vector; the host sums the 8 partials.

Device pipeline: the host pre-transposes and pre-stacks M into the two
SBUF layouts the PE needs (lhsT = M^T 2-stacked across partition halves,
rhs = M stacked the same way), both bf16, interleaved into ONE dram
tensor.  DMA triggers carry a ~600ns fixed cost on the issuing engine,
so the input moves as 16 half-super-chunk DMAs (512KB, 4KB/partition
lines = 128 descriptors each) triggered by Sync from instruction 0; the
small tail constants go through GpSimd's SWDGE so they never delay the
pipeline.  Compute per quarter-super-chunk (16 pairs): one single-bank
PSUM tile takes 16 single-pass bf16 matmuls; rowsums run on DVE
(tensor_reduce, 3 quarters per super-chunk) and ACT (activation
accum_out per d-slot, the 4th quarter) so neither engine paces the
DMA-bound loop.  The V-transpose tail is interleaved with the loop per
128-pair half-run; the final weighted sum is matmul(lhsT=u, rhs=vt) so
the result lands as [1, 64] and the output DMA is one descriptor.
"""

import numpy as np

_N = 64
_NCORES = 8
_NP = 512          # pairs per core
_DG = 32           # d-slots per super-chunk (pairs per super-chunk = 2*_DG)
_NG = _NP // (2 * _DG)   # 8 super-chunks
_DH = _DG // 2     # d-slots per half-DMA
_DQ = _DG // 4     # d-slots per PSUM quarter

_cached = {}


def _build_program():
    import concourse.tile as tile
    from concourse import bacc, mybir
    from contextlib import ExitStack

    f32 = mybir.dt.float32
    bf16 = mybir.dt.bfloat16
    nc = bacc.Bacc("TRN2", target_bir_lowering=False, debug=False)
    mm_in = nc.dram_tensor("mm", [_NG, 2, 2, 128, 2, _DQ, _N], bf16,
                           kind="ExternalInput").ap()
    mask_in = nc.dram_tensor("mask", [128, 4, _N], f32, kind="ExternalInput").ap()
    xpp_in = nc.dram_tensor("xpp", [128, 4], f32, kind="ExternalInput").ap()
    wpp_in = nc.dram_tensor("wpp", [128, 4], f32, kind="ExternalInput").ap()
    rpp_in = nc.dram_tensor("rpp", [128, 4], f32, kind="ExternalInput").ap()
    ident_in = nc.dram_tensor("ident", [_N, _N], f32, kind="ExternalInput").ap()
    out_dram = nc.dram_tensor("out", [1, _N], f32, kind="ExternalOutput").ap()

    with tile.TileContext(nc) as tc:
        with ExitStack() as ctx:
            consts = ctx.enter_context(tc.tile_pool(name="consts", bufs=1))
            work = ctx.enter_context(tc.tile_pool(name="work", bufs=6))
            psum = ctx.enter_context(tc.tile_pool(name="psum", bufs=6, space="PSUM"))
            psumt = ctx.enter_context(tc.tile_pool(name="psumt", bufs=1, space="PSUM"))

            # tail constants via GpSimd SWDGE: keeps the HWDGE engines free
            # to launch the big input DMAs immediately
            ident = consts.tile([_N, _N], f32)
            nc.gpsimd.dma_start(out=ident[:, :], in_=ident_in[:, :])
            mask_sb = consts.tile([128, 4, _N], f32)
            nc.gpsimd.dma_start(out=mask_sb[:, :, :], in_=mask_in[:, :, :])
            xpp_sb = consts.tile([128, 4], f32)
            nc.gpsimd.dma_start(out=xpp_sb[:, :], in_=xpp_in[:, :])
            wpp_sb = consts.tile([128, 4], f32)
            nc.gpsimd.dma_start(out=wpp_sb[:, :], in_=wpp_in[:, :])
            rpp_sb = consts.tile([128, 4], f32)
            nc.gpsimd.dma_start(out=rpp_sb[:, :], in_=rpp_in[:, :])
            v_sb = consts.tile([128, _NG, _DG], f32)
            act_dummy = consts.tile([128, _N], bf16)
            # tpp = x*W_t*r_diag does not depend on V: compute it up front
            tpp = consts.tile([128, 4], f32)
            nc.gpsimd.tensor_mul(out=tpp[:, :], in0=xpp_sb[:, :], in1=wpp_sb[:, :])
            nc.gpsimd.tensor_mul(out=tpp[:, :], in0=tpp[:, :], in1=rpp_sb[:, :])

            v64 = consts.tile([_N, 2, _NP // 2], f32)
            pvt = psumt.tile([128, 4, _N], f32, tag="pvt")
            vt = consts.tile([128, 4, _N], f32)

            v_flat = v_sb[:, :, :].rearrange("p a b -> p (a b)")  # [128, 256]

            maskv = consts.tile([128, 4, _N], f32)
            d_sb = consts.tile([128, 4], f32)
            dinv = consts.tile([128, 4], f32)
            u = consts.tile([128, 4], f32)
            prbc = psumt.tile([1, _N], f32, tag="prbc")

            for g in range(_NG):
                mm = work.tile([128, 2, 2, 2, _DQ, _N], bf16, tag="mm")
                for hd in (0, 1):
                    for q in (0, 1):
                        # quarter-granularity DMAs (2KB/partition lines, 128
                        # descriptors each) so compute is gated on 0.7us of
                        # transfer instead of 1.45us at every boundary
                        trig = nc.sync if hd == 0 else nc.scalar
                        trig.dma_start(out=mm[:, hd, q, :, :, :],
                                       in_=mm_in[g, hd, q, :, :, :, :])
                for hd in (0, 1):
                    for q in (0, 1):
                        pp = psum.tile([128, _DQ, _N], f32, tag="pp")
                        for dd in range(_DQ):
                            for h in (0, 1):
                                b = 64 * h
                                nc.tensor.matmul(
                                    out=pp[b:b + 64, dd, :],
                                    lhsT=mm[b:b + 64, hd, q, 0, dd, :],
                                    rhs=mm[b:b + 64, hd, q, 1, dd, :],
                                    start=True, stop=True)
                        d0 = _DH * hd + _DQ * q
                        nc.vector.tensor_reduce(
                            out=v_sb[:, g, d0:d0 + _DQ],
                            in_=pp[:, :, :],
                            axis=mybir.AxisListType.X, op=mybir.AluOpType.add)

                if g == 3:
                    # V rows of the finished first 128-pair half-run to
                    # partitions 0-63 on the otherwise-idle GpSimd engine
                    # (keeps the DMA-trigger engines' queues unblocked)
                    sl = slice(0, 128)
                    nc.gpsimd.tensor_copy(out=v64[:, 0, sl], in_=v_flat[0:64, sl])
                    nc.gpsimd.tensor_copy(out=v64[:, 1, sl], in_=v_flat[64:128, sl])
                if g == 4:
                    # 2 (0,0) double-transposes of the first half-run; the
                    # PSUM->SBUF copies go on ACT one super-chunk later than
                    # the copies above so they never stall a trigger
                    sl = slice(0, 128)
                    for h in (0, 1):
                        j = 2 * h
                        nc.tensor.transpose(
                            out=pvt[:, j, :],
                            in_=v64[:, h, sl],
                            identity=ident[:, :])
                        nc.scalar.copy(out=vt[:, j, :], in_=pvt[:, j, :])
                if g == 5:
                    # first half-run's denominator gather + weighted-sum
                    # matmuls, emitted 2 super-chunks later so every input is
                    # long ready and no engine queue stalls.  Denominators
                    # via VT-layout mask gather on DVE (exact fp32; PE
                    # ones-matmuls with 128-wide lhsT lose ~11 bits).
                    jj = slice(0, 4, 2)   # VT slots {0, 2}
                    nc.vector.tensor_mul(out=maskv[:, jj, :], in0=vt[:, jj, :],
                                         in1=mask_sb[:, jj, :])
                    nc.vector.tensor_reduce(
                        out=d_sb[:, jj], in_=maskv[:, jj, :],
                        axis=mybir.AxisListType.X, op=mybir.AluOpType.add)
                    nc.vector.reciprocal(out=dinv[:, jj], in_=d_sb[:, jj])
                    nc.vector.tensor_mul(out=u[:, jj], in0=tpp[:, jj],
                                         in1=dinv[:, jj])
                    # rbc[n] = sum_{q,j} u[q,j] vt[q,j,n]: u stationary,
                    # [1,64] out so the output DMA is a single descriptor
                    for h in (0, 1):
                        nc.tensor.matmul(
                            out=prbc[:, :], lhsT=u[:, 2 * h:2 * h + 1],
                            rhs=vt[:, 2 * h, :],
                            start=(h == 0), stop=False)

            # ---- tail: second half-run's V chain, then finish ----
            sl = slice(128, 256)
            nc.scalar.copy(out=v64[:, 0, sl], in_=v_flat[0:64, sl])
            nc.scalar.copy(out=v64[:, 1, sl], in_=v_flat[64:128, sl])
            for h in (0, 1):
                j = 2 * h + 1
                nc.tensor.transpose(
                    out=pvt[:, j, :],
                    in_=v64[:, h, sl],
                    identity=ident[:, :])
                nc.scalar.copy(out=vt[:, j, :], in_=pvt[:, j, :])
            jj = slice(1, 4, 2)   # VT slots {1, 3}
            nc.vector.tensor_mul(out=maskv[:, jj, :], in0=vt[:, jj, :],
                                 in1=mask_sb[:, jj, :])
            nc.vector.tensor_reduce(
                out=d_sb[:, jj], in_=maskv[:, jj, :],
                axis=mybir.AxisListType.X, op=mybir.AluOpType.add)
            nc.vector.reciprocal(out=dinv[:, jj], in_=d_sb[:, jj])
            nc.vector.tensor_mul(out=u[:, jj], in0=tpp[:, jj], in1=dinv[:, jj])
            for h in (0, 1):
                j = 2 * h + 1
                nc.tensor.matmul(
                    out=prbc[:, :], lhsT=u[:, j:j + 1], rhs=vt[:, j, :],
                    start=False, stop=(h == 1))
            out_sb = consts.tile([1, _N], f32)
            nc.vector.tensor_copy(out=out_sb[:, :], in_=prbc[:, :])
            nc.sync.dma_start(out=out_dram[:, :], in_=out_sb[:, :])
    nc.compile()
    return nc


def _get_program():
    if "nc" not in _cached:
        _cached["nc"] = _build_program()
    return _cached["nc"]


def _pair_of(h, f):
    """Local pair id for half h, V-free-index f (f = _DG*g + dslot)."""
    return 2 * _DG * (f // _DG) + 2 * (f % _DG) + h


def _host_layouts(x, weights_t, r_const):
    """Per-core gathers: xpp/wpp/rpp [128,4] pairs-on-partitions, mask [128,4,64]."""
    outs = []
    for c in range(_NCORES):
        # VT layout: vt[q, j, i] = v_pair[i], pair = _pair_of(j>>1, 128*(j&1)+q)
        Q = np.arange(128)[:, None]
        J = np.arange(4)[None, :]
        h = J >> 1
        g = J & 1
        f = 128 * g + Q
        p = _pair_of(h, f)                      # local pair id [128, 4]
        s_loc = p >> 6
        t = p & 63
        s_glob = 8 * c + s_loc
        xpp = np.ascontiguousarray(x[s_glob, t], np.float32)
        wpp = np.ascontiguousarray(weights_t[s_glob, t], np.float32)
        rpp = np.ascontiguousarray(r_const[s_glob, t, s_glob, s_glob], np.float32)
        # mask[q, j, i] = 1 iff i == s_glob(pair at VT position (q, j))
        mask = np.zeros((128, 4, _N), np.float32)
        for j in range(4):
            hh = j >> 1
            ff = 128 * (j & 1) + np.arange(128)
            pl = _pair_of(hh, ff)
            sg = 8 * c + (pl >> 6)
            mask[np.arange(128), j, sg] = 1.0
        outs.append((xpp, wpp, rpp, mask))
    return outs


def _device_m_layouts(M_core):
    """M_core [512, 64, 64] f32 -> mm [NG, 2, 2, 128, 2, DQ, 64] bf16.

    d = DH*hd + DQ*q + dd;  p = 2*DG*g + 2*d + h
    mm[g, hd, q, 64h+j, 0, dd, i] = M[p][i, j]   (lhsT = M^T, 2-stacked)
    mm[g, hd, q, 64h+j, 1, dd, m] = M[p][j, m]   (rhs  = M,   2-stacked)
    """
    import ml_dtypes
    a = M_core.reshape(_NG, 2, 2, _DQ, 2, _N, _N)    # [g, hd, q, dd, h, i, j]
    mt = a.transpose(0, 1, 2, 4, 6, 3, 5)            # [g, hd, q, h, j, dd, i]
    mc = a.transpose(0, 1, 2, 4, 5, 3, 6)            # [g, hd, q, h, j, dd, m]
    mm = np.stack([mt.reshape(_NG, 2, 2, 128, _DQ, _N),
                   mc.reshape(_NG, 2, 2, 128, _DQ, _N)], axis=4)
    return np.ascontiguousarray(mm).astype(ml_dtypes.bfloat16)


def kernel(x, weights_t, weights_r, r_zeros, r_const):
    from concourse.bass_utils import run_bass_kernel_spmd

    x = np.asarray(x, np.float32)
    weights_t = np.asarray(weights_t, np.float32)
    r_const = np.asarray(r_const, np.float32)
    r_zeros_np = np.asarray(r_zeros)
    if np.any(r_zeros_np):
        M_all = (np.asarray(weights_r, np.float32) * r_zeros_np.astype(np.float32)
                 + r_const).reshape(_N * _N, _N, _N)
    else:
        M_all = r_const.reshape(_N * _N, _N, _N)

    nc = _get_program()
    ident_np = np.eye(_N, dtype=np.float32)
    layouts = _host_layouts(x, weights_t, r_const)
    in_maps = []
    for c in range(_NCORES):
        xpp, wpp, rpp, mask = layouts[c]
        in_maps.append({
            "mm": _device_m_layouts(M_all[_NP * c:_NP * (c + 1)]),
            "mask": mask,
            "xpp": xpp,
            "wpp": wpp,
            "rpp": rpp,
            "ident": ident_np,
        })
    res = run_bass_kernel_spmd(nc, in_maps, core_ids=list(range(_NCORES)))
    parts = np.stack([r["out"][0, :] for r in res.results])  # [8, 64]
    return parts.sum(axis=0, dtype=np.float64).astype(np.float32)


# revision 23
# speedup vs baseline: 1.0984x; 1.0984x over previous
"""Trainium2 Bass kernel for nn_DegreePrediction (RBC via batched Perron vectors).

Math: M[s,t] = weights_r*r_zeros + r_const is positive column-stochastic
(columns sum to 1), so its eigenvalue-1 right eigenvector is the Perron
vector, and the reference formula  rbc[n] = sum_{s,t} T[s,t]/v[s,t,s] * v[s,t,n]
is invariant to the scale of v.  rowsums(M^2) = M^2 @ 1 converges to v at
rate lambda2^2 (lambda2 <= ~0.09 here); in bf16 the end-to-end error is
~1.6e-3, well inside the 2e-2 gate, so ONE bf16 64x64x64 matmul per pair
suffices.

Sharding: the 4096 (s,t) pairs are split by s across 8 cores (512 pairs
each).  Each core computes a partial 64-vector; the host sums the 8 partials.

Device pipeline: the host pre-transposes and pre-stacks M into the two
SBUF layouts the PE needs (lhsT = M^T 2-stacked across partition halves,
rhs = M stacked the same way), both bf16, interleaved into ONE dram
tensor.  DMA triggers carry a ~600ns fixed cost on the issuing engine,
so the input moves as 16 half-super-chunk DMAs (512KB, 4KB/partition
lines = 128 descriptors each) triggered by Sync from instruction 0; the
small tail constants go through GpSimd's SWDGE so they never delay the
pipeline.  Compute per quarter-super-chunk (16 pairs): one single-bank
PSUM tile takes 16 single-pass bf16 matmuls; rowsums run on DVE
(tensor_reduce, 3 quarters per super-chunk) and ACT (activation
accum_out per d-slot, the 4th quarter) so neither engine paces the
DMA-bound loop.  The V-transpose tail is interleaved with the loop per
128-pair half-run; the final weighted sum is matmul(lhsT=u, rhs=vt) so
the result lands as [1, 64] and the output DMA is one descriptor.
"""

import numpy as np

_N = 64
_NCORES = 8
_NP = 512          # pairs per core
_DG = 32           # d-slots per super-chunk (pairs per super-chunk = 2*_DG)
_NG = _NP // (2 * _DG)   # 8 super-chunks
_DH = _DG // 2     # d-slots per half-DMA
_DQ = _DG // 4     # d-slots per PSUM quarter

_cached = {}


def _build_program():
    import concourse.tile as tile
    from concourse import bacc, mybir
    from contextlib import ExitStack

    f32 = mybir.dt.float32
    bf16 = mybir.dt.bfloat16
    nc = bacc.Bacc("TRN2", target_bir_lowering=False, debug=False)
    mm_in = nc.dram_tensor("mm", [_NG, 2, 128, 2, _DH, _N], bf16,
                           kind="ExternalInput").ap()
    mask_in = nc.dram_tensor("mask", [128, 4, _N], f32, kind="ExternalInput").ap()
    xpp_in = nc.dram_tensor("xpp", [128, 4], f32, kind="ExternalInput").ap()
    wpp_in = nc.dram_tensor("wpp", [128, 4], f32, kind="ExternalInput").ap()
    rpp_in = nc.dram_tensor("rpp", [128, 4], f32, kind="ExternalInput").ap()
    ident_in = nc.dram_tensor("ident", [_N, _N], f32, kind="ExternalInput").ap()
    out_dram = nc.dram_tensor("out", [1, _N], f32, kind="ExternalOutput").ap()

    with tile.TileContext(nc) as tc:
        with ExitStack() as ctx:
            consts = ctx.enter_context(tc.tile_pool(name="consts", bufs=1))
            work = ctx.enter_context(tc.tile_pool(name="work", bufs=6))
            psum = ctx.enter_context(tc.tile_pool(name="psum", bufs=6, space="PSUM"))
            psumt = ctx.enter_context(tc.tile_pool(name="psumt", bufs=1, space="PSUM"))

            # tail constants via GpSimd SWDGE: keeps the HWDGE engines free
            # to launch the big input DMAs immediately
            ident = consts.tile([_N, _N], f32)
            nc.gpsimd.dma_start(out=ident[:, :], in_=ident_in[:, :])
            mask_sb = consts.tile([128, 4, _N], f32)
            nc.gpsimd.dma_start(out=mask_sb[:, :, :], in_=mask_in[:, :, :])
            xpp_sb = consts.tile([128, 4], f32)
            nc.gpsimd.dma_start(out=xpp_sb[:, :], in_=xpp_in[:, :])
            wpp_sb = consts.tile([128, 4], f32)
            nc.gpsimd.dma_start(out=wpp_sb[:, :], in_=wpp_in[:, :])
            rpp_sb = consts.tile([128, 4], f32)
            nc.gpsimd.dma_start(out=rpp_sb[:, :], in_=rpp_in[:, :])
            v_sb = consts.tile([128, _NG, _DG], f32)
            act_dummy = consts.tile([128, _N], bf16)
            # tpp = x*W_t*r_diag does not depend on V: compute it up front
            tpp = consts.tile([128, 4], f32)
            nc.gpsimd.tensor_mul(out=tpp[:, :], in0=xpp_sb[:, :], in1=wpp_sb[:, :])
            nc.gpsimd.tensor_mul(out=tpp[:, :], in0=tpp[:, :], in1=rpp_sb[:, :])

            v64 = consts.tile([_N, 2, _NP // 2], f32)
            pvt = psumt.tile([128, 4, _N], f32, tag="pvt")
            vt = consts.tile([128, 4, _N], f32)

            v_flat = v_sb[:, :, :].rearrange("p a b -> p (a b)")  # [128, 256]

            maskv = consts.tile([128, 4, _N], f32)
            d_sb = consts.tile([128, 4], f32)
            dinv = consts.tile([128, 4], f32)
            u = consts.tile([128, 4], f32)
            prbc = psumt.tile([1, _N], f32, tag="prbc")

            for g in range(_NG):
                mm = work.tile([128, 2, 2, _DH, _N], bf16, tag="mm")
                if g == 0:
                    # quarter-granularity DMAs for the first super-chunk so
                    # the PE starts ~1us earlier
                    for hd in (0, 1):
                        for q in (0, 1):
                            trig = nc.sync if hd == 0 else nc.scalar
                            trig.dma_start(
                                out=mm[:, hd, :, _DQ * q:_DQ * (q + 1), :],
                                in_=mm_in[g, hd, :, :, _DQ * q:_DQ * (q + 1), :])
                else:
                    for hd in (0, 1):
                        trig = nc.sync if hd == 0 else nc.scalar
                        trig.dma_start(out=mm[:, hd, :, :, :],
                                       in_=mm_in[g, hd, :, :, :, :])
                for hd in (0, 1):
                    for q in (0, 1):
                        pp = psum.tile([128, _DQ, _N], f32, tag="pp")
                        for dd in range(_DQ):
                            for h in (0, 1):
                                b = 64 * h
                                nc.tensor.matmul(
                                    out=pp[b:b + 64, dd, :],
                                    lhsT=mm[b:b + 64, hd, 0, _DQ * q + dd, :],
                                    rhs=mm[b:b + 64, hd, 1, _DQ * q + dd, :],
                                    start=True, stop=True)
                        d0 = _DH * hd + _DQ * q
                        nc.vector.tensor_reduce(
                            out=v_sb[:, g, d0:d0 + _DQ],
                            in_=pp[:, :, :],
                            axis=mybir.AxisListType.X, op=mybir.AluOpType.add)

                if g == 3:
                    # V rows of the finished first 128-pair half-run to
                    # partitions 0-63, then 2 (0,0) double-transposes,
                    # overlapped with the next super-chunks' compute
                    sl = slice(0, 128)
                    nc.scalar.copy(out=v64[:, 0, sl], in_=v_flat[0:64, sl])
                    nc.scalar.copy(out=v64[:, 1, sl], in_=v_flat[64:128, sl])
                    for h in (0, 1):
                        j = 2 * h
                        nc.tensor.transpose(
                            out=pvt[:, j, :],
                            in_=v64[:, h, sl],
                            identity=ident[:, :])
                        nc.scalar.copy(out=vt[:, j, :], in_=pvt[:, j, :])
                if g == 5:
                    # first half-run's denominator gather + weighted-sum
                    # matmuls, emitted 2 super-chunks later so every input is
                    # long ready and no engine queue stalls.  Denominators
                    # via VT-layout mask gather on DVE (exact fp32; PE
                    # ones-matmuls with 128-wide lhsT lose ~11 bits).
                    jj = slice(0, 4, 2)   # VT slots {0, 2}
                    nc.vector.tensor_mul(out=maskv[:, jj, :], in0=vt[:, jj, :],
                                         in1=mask_sb[:, jj, :])
                    nc.vector.tensor_reduce(
                        out=d_sb[:, jj], in_=maskv[:, jj, :],
                        axis=mybir.AxisListType.X, op=mybir.AluOpType.add)
                    nc.vector.reciprocal(out=dinv[:, jj], in_=d_sb[:, jj])
                    nc.vector.tensor_mul(out=u[:, jj], in0=tpp[:, jj],
                                         in1=dinv[:, jj])
                    # rbc[n] = sum_{q,j} u[q,j] vt[q,j,n]: u stationary,
                    # [1,64] out so the output DMA is a single descriptor
                    for h in (0, 1):
                        nc.tensor.matmul(
                            out=prbc[:, :], lhsT=u[:, 2 * h:2 * h + 1],
                            rhs=vt[:, 2 * h, :],
                            start=(h == 0), stop=False)

            # ---- tail: second half-run's V chain, then finish ----
            sl = slice(128, 256)
            nc.scalar.copy(out=v64[:, 0, sl], in_=v_flat[0:64, sl])
            nc.scalar.copy(out=v64[:, 1, sl], in_=v_flat[64:128, sl])
            for h in (0, 1):
                j = 2 * h + 1
                nc.tensor.transpose(
                    out=pvt[:, j, :],
                    in_=v64[:, h, sl],
                    identity=ident[:, :])
                nc.scalar.copy(out=vt[:, j, :], in_=pvt[:, j, :])
            jj = slice(1, 4, 2)   # VT slots {1, 3}
            nc.vector.tensor_mul(out=maskv[:, jj, :], in0=vt[:, jj, :],
                                 in1=mask_sb[:, jj, :])
            nc.vector.tensor_reduce(
                out=d_sb[:, jj], in_=maskv[:, jj, :],
                axis=mybir.AxisListType.X, op=mybir.AluOpType.add)
            nc.vector.reciprocal(out=dinv[:, jj], in_=d_sb[:, jj])
            nc.vector.tensor_mul(out=u[:, jj], in0=tpp[:, jj], in1=dinv[:, jj])
            for h in (0, 1):
                j = 2 * h + 1
                nc.tensor.matmul(
                    out=prbc[:, :], lhsT=u[:, j:j + 1], rhs=vt[:, j, :],
                    start=False, stop=(h == 1))
            out_sb = consts.tile([1, _N], f32)
            nc.vector.tensor_copy(out=out_sb[:, :], in_=prbc[:, :])
            nc.sync.dma_start(out=out_dram[:, :], in_=out_sb[:, :])
    nc.compile()
    return nc


def _get_program():
    if "nc" not in _cached:
        _cached["nc"] = _build_program()
    return _cached["nc"]


def _pair_of(h, f):
    """Local pair id for half h, V-free-index f (f = _DG*g + dslot)."""
    return 2 * _DG * (f // _DG) + 2 * (f % _DG) + h


def _host_layouts(x, weights_t, r_const):
    """Per-core gathers: xpp/wpp/rpp [128,4] pairs-on-partitions, mask [128,4,64]."""
    outs = []
    for c in range(_NCORES):
        # VT layout: vt[q, j, i] = v_pair[i], pair = _pair_of(j>>1, 128*(j&1)+q)
        Q = np.arange(128)[:, None]
        J = np.arange(4)[None, :]
        h = J >> 1
        g = J & 1
        f = 128 * g + Q
        p = _pair_of(h, f)                      # local pair id [128, 4]
        s_loc = p >> 6
        t = p & 63
        s_glob = 8 * c + s_loc
        xpp = np.ascontiguousarray(x[s_glob, t], np.float32)
        wpp = np.ascontiguousarray(weights_t[s_glob, t], np.float32)
        rpp = np.ascontiguousarray(r_const[s_glob, t, s_glob, s_glob], np.float32)
        # mask[q, j, i] = 1 iff i == s_glob(pair at VT position (q, j))
        mask = np.zeros((128, 4, _N), np.float32)
        for j in range(4):
            hh = j >> 1
            ff = 128 * (j & 1) + np.arange(128)
            pl = _pair_of(hh, ff)
            sg = 8 * c + (pl >> 6)
            mask[np.arange(128), j, sg] = 1.0
        outs.append((xpp, wpp, rpp, mask))
    return outs


def _device_m_layouts(M_core):
    """M_core [512, 64, 64] f32 -> mm [NG, 2, 128, 2, DH, 64] bf16.

    d = DH*hd + dd;  p = 2*DG*g + 2*d + h
    mm[g, hd, 64h+j, 0, dd, i] = M[p][i, j]   (lhsT = M^T, 2-stacked)
    mm[g, hd, 64h+j, 1, dd, m] = M[p][j, m]   (rhs  = M,   2-stacked)
    """
    import ml_dtypes
    a = M_core.reshape(_NG, 2, _DH, 2, _N, _N)       # [g, hd, dd, h, i, j]
    mt = a.transpose(0, 1, 3, 5, 2, 4)               # [g, hd, h, j, dd, i]
    mc = a.transpose(0, 1, 3, 4, 2, 5)               # [g, hd, h, j, dd, m]
    mm = np.stack([mt.reshape(_NG, 2, 128, _DH, _N),
                   mc.reshape(_NG, 2, 128, _DH, _N)], axis=3)
    return np.ascontiguousarray(mm).astype(ml_dtypes.bfloat16)


def kernel(x, weights_t, weights_r, r_zeros, r_const):
    from concourse.bass_utils import run_bass_kernel_spmd

    x = np.asarray(x, np.float32)
    weights_t = np.asarray(weights_t, np.float32)
    r_const = np.asarray(r_const, np.float32)
    r_zeros_np = np.asarray(r_zeros)
    if np.any(r_zeros_np):
        M_all = (np.asarray(weights_r, np.float32) * r_zeros_np.astype(np.float32)
                 + r_const).reshape(_N * _N, _N, _N)
    else:
        M_all = r_const.reshape(_N * _N, _N, _N)

    nc = _get_program()
    ident_np = np.eye(_N, dtype=np.float32)
    layouts = _host_layouts(x, weights_t, r_const)
    in_maps = []
    for c in range(_NCORES):
        xpp, wpp, rpp, mask = layouts[c]
        in_maps.append({
            "mm": _device_m_layouts(M_all[_NP * c:_NP * (c + 1)]),
            "mask": mask,
            "xpp": xpp,
            "wpp": wpp,
            "rpp": rpp,
            "ident": ident_np,
        })
    res = run_bass_kernel_spmd(nc, in_maps, core_ids=list(range(_NCORES)))
    parts = np.stack([r["out"][0, :] for r in res.results])  # [8, 64]
    return parts.sum(axis=0, dtype=np.float64).astype(np.float32)
